# revision 19
# baseline (speedup 1.0000x reference)
"""SAN aggregation kernel for Trainium2 (Bass/Tile), 8-core data-parallel.

Problem: out[n,c,h,w] = sum_k w[n, c//8, k, h*W+w] * xpad[n, c, h+dh(k), w+dw(k)]
  x: [8, 64, 128, 128] f32, w: [8, 8, 9, 16384] f32, 3x3 window, pad 1.

Sharding: batch dim N=8 across 8 NeuronCores (1 image per core, no
cross-core communication).

Per-core layout (everything resident in SBUF):
  partitions p = hb*8 + cw   (hb: 16 row-blocks of 8 rows, cw: 8 weight chans)
  x_sb  [128, 8*10*130]: per (gl): rows [hb*8-1, hb*8+9) of channel
        c=cw*8+gl, each row stored with 130-px pitch (1 left + 128 + 1 right
        zero pad).  Shift (dh,dw) == flat offset dh*130+dw.
  w_sb  [128, 9*1024]:  w[cw, k, hb-rows] per partition, k-major.

Compute: per gl, one (or nine) DVE tensor_mul producing all 9 k-product
planes into a q tile, then the 9-way k-reduction either on DVE
(tensor_reduce axis=X, k-innermost q layout) or on GPSIMD (pairwise
tensor_add tree, k-major q layout).  GPSIMD runs concurrently with DVE
(tensor ops use only DVE's dedicated SBUF port pair).
"""

import sys
import os

for _p in ("/opt/trn_rl_repo", "/root/.axon_site/_ro/trn_rl_repo"):
    if _p not in sys.path and os.path.isdir(_p):
        sys.path.append(_p)

import numpy as np

import concourse.bass as bass
import concourse.bacc as bacc
import concourse.mybir as mybir
import bass_rust
from concourse.tile import TileContext

F32 = mybir.dt.float32

C, H, W = 64, 128, 128
S = H * W          # 16384
CW, GL = 8, 8      # weight channels, share planes
HB = 16            # row blocks
RB = H // HB       # rows per block = 8
PITCH = W + 2      # 130
XROWS = RB + 2     # 10 rows incl halo
XGL = XROWS * PITCH  # 1300 elements per gl block in x_sb
SB = RB * W        # 1024 spatial elems per partition per gl

# gls whose k-reduction runs on DVE (tensor_reduce); the rest use a GPSIMD
# add tree.  DVE-reduce gls are placed last so GPSIMD ramps up first.
N_DVE_REDUCE = 2


def _ap(base, dims, extra_offset=0):
    """Copy AP `base`, replace its [step,count] dims, bump offset.

    dims[0] is the partition dim: step "P" substitutes the base AP's own
    partition stride (flat element space, = free width).
    """
    c = base.copy()
    pstep = base.ap[0][0]
    dims = [[pstep if s == "P" else s, n] for s, n in dims]
    c.ap = bass_rust.VecI64Pair(dims)
    if extra_offset:
        c.offset = c.offset + extra_offset
    return c


def build_program(n_dve_reduce=N_DVE_REDUCE):
    nc = bacc.Bacc("TRN2", target_bir_lowering=False, debug=False)
    x_d = nc.dram_tensor("x", [C, S], F32, kind="ExternalInput")
    w_d = nc.dram_tensor("w", [CW, 9, S], F32, kind="ExternalInput")
    o_d = nc.dram_tensor("out", [C, S], F32, kind="ExternalOutput")

    with TileContext(nc) as tc:
        with tc.tile_pool(name="main", bufs=1) as pool, \
             tc.tile_pool(name="qs", bufs=2) as qpool, \
             tc.tile_pool(name="stg", bufs=3) as spool, \
             tc.tile_pool(name="os", bufs=8) as opool:
            x_sb = pool.tile([128, GL * XGL], F32)
            w_sb = pool.tile([128, 9 * SB], F32)

            # ---- zero x_sb padding (left/right cols, halo rows).  Halo
            # rows zeroed on ALL partitions; r=0/r=9 DMAs overwrite the
            # valid parts (Tile orders the WAW deps).
            nc.gpsimd.memset(
                _ap(x_sb[:], [["P", 128], [XGL, GL], [PITCH, RB],
                              [PITCH - 1, 2]], extra_offset=PITCH), 0.0)
            nc.gpsimd.memset(
                _ap(x_sb[:], [["P", 128], [XGL, GL], [1, PITCH]]), 0.0)
            nc.gpsimd.memset(
                _ap(x_sb[:], [["P", 128], [XGL, GL], [1, PITCH]],
                    extra_offset=(XROWS - 1) * PITCH), 0.0)

            # ---- x load: per gl, big contiguous HBM DMAs into a
            # 128-pitch staging tile (stage[p, r*128+j] = x row hb*8-1+r),
            # then one SBUF->SBUF DMA re-layout into the 130-pitch padded
            # x_sb block.  Big DMAs keep HBM near peak bandwidth.
            def load_w_k(k):
                nc.sync.dma_start(
                    out=_ap(w_sb[:], [["P", 128], [1, SB]],
                            extra_offset=k * SB),
                    in_=_ap(w_d.ap(), [[SB, HB], [9 * S, CW], [1, SB]],
                            extra_offset=k * S))

            def load_x_gl(gl, stage):
                # main: partitions 8..120 (hb 1..14), rows hb*8-1 .. hb*8+9
                # = 1280 contiguous elements of channel c starting at
                # (hb*8-1)*128.
                nc.sync.dma_start(
                    out=_ap(stage[8:120], [["P", 112], [1, XROWS * W]]),
                    in_=_ap(x_d.ap(), [[RB * W, HB - 2], [GL * S, CW],
                                       [1, XROWS * W]],
                            extra_offset=gl * S + (RB - 1) * W))
                # hb=0 (partitions 0..8): stage rows r=1..9 = x rows 0..8
                nc.scalar.dma_start(
                    out=_ap(stage[0:8], [["P", 8], [1, (XROWS - 1) * W]],
                            extra_offset=W),
                    in_=_ap(x_d.ap(), [[GL * S, CW], [1, (XROWS - 1) * W]],
                            extra_offset=gl * S))
                # hb=15 (partitions 120..128): stage rows r=0..8 = x rows
                # 119..127
                nc.scalar.dma_start(
                    out=_ap(stage[120:128], [["P", 8], [1, (XROWS - 1) * W]]),
                    in_=_ap(x_d.ap(), [[GL * S, CW], [1, (XROWS - 1) * W]],
                            extra_offset=gl * S + (H - XROWS + 1) * W))
                # re-layout 128-pitch -> 130-pitch (cols 1..129), skipping
                # the zeroed halo rows at the hb=0 / hb=15 edges.
                nc.scalar.dma_start(
                    out=_ap(x_sb[8:120], [["P", 112], [PITCH, XROWS], [1, W]],
                            extra_offset=gl * XGL + 1),
                    in_=_ap(stage[8:120], [["P", 112], [W, XROWS], [1, W]]))
                nc.scalar.dma_start(
                    out=_ap(x_sb[0:8], [["P", 8], [PITCH, XROWS - 1], [1, W]],
                            extra_offset=gl * XGL + PITCH + 1),
                    in_=_ap(stage[0:8], [["P", 8], [W, XROWS - 1], [1, W]],
                            extra_offset=W))
                nc.scalar.dma_start(
                    out=_ap(x_sb[120:128], [["P", 8], [PITCH, XROWS - 1],
                                            [1, W]],
                            extra_offset=gl * XGL + 1),
                    in_=_ap(stage[120:128], [["P", 8], [W, XROWS - 1],
                                             [1, W]]))

            load_w_k(0)
            load_w_k(1)
            stages = []
            for gl in range(GL):
                stage = spool.tile([128, XROWS * W], F32, tag="stage")
                load_x_gl(gl, stage)
                if gl == 0:
                    for k in range(2, 9):
                        load_w_k(k)

            # ---- compute ----
            def xv(gl, dh, dw):
                return _ap(x_sb[:], [["P", 128], [PITCH, RB], [1, W]],
                           extra_offset=gl * XGL + dh * PITCH + dw)

            def mult_all_k(gl, q, kmajor):
                """Nine tensor_muls producing the 9 k-product planes.
                (DVE ISA APs are TENSOR3D: partition + 2 free dims max.)"""
                for k in range(9):
                    dh, dw = divmod(k, 3)
                    wv = _ap(w_sb[:], [["P", 128], [W, RB], [1, W]],
                             extra_offset=k * SB)
                    if kmajor:   # q[p, k*SB + s]
                        qv = _ap(q[:], [["P", 128], [W, RB], [1, W]],
                                 extra_offset=k * SB)
                    else:        # q[p, s*9 + k]
                        qv = _ap(q[:], [["P", 128], [9 * W, RB], [9, W]],
                                 extra_offset=k)
                    nc.vector.tensor_mul(out=qv, in0=xv(gl, dh, dw), in1=wv)

            def out_dma(gl, src):
                nc.scalar.dma_start(
                    out=_ap(o_d.ap(), [[RB * W, HB], [GL * S, CW], [1, SB]],
                            extra_offset=gl * S),
                    in_=src)

            # All compute on DVE: GPSIMD streaming concurrently with DVE
            # measurably slows DVE ~2.6x (shared SBUF port), so offloading
            # the reduction is a net loss.  Per gl: 9 muls + 8 adds, then
            # the output DMA (per-gl so stores overlap later compute).
            for gl in range(GL):
                acc_t = opool.tile([128, SB], F32, tag="o")
                acc = _ap(acc_t[:], [["P", 128], [W, RB], [1, W]])
                for k in range(9):
                    dh, dw = divmod(k, 3)
                    wv = _ap(w_sb[:], [["P", 128], [W, RB], [1, W]],
                             extra_offset=k * SB)
                    if k == 0:
                        nc.vector.tensor_mul(out=acc, in0=xv(gl, dh, dw),
                                             in1=wv)
                    else:
                        tmp = qpool.tile([128, SB], F32, tag="tmp")
                        t = _ap(tmp[:], [["P", 128], [W, RB], [1, W]])
                        nc.vector.tensor_mul(out=t, in0=xv(gl, dh, dw),
                                             in1=wv)
                        nc.vector.tensor_add(out=acc, in0=acc, in1=t)
                out_dma(gl, _ap(acc_t[:], [["P", 128], [1, SB]]))

    nc.compile()
    return nc


_NC_CACHE = None


def _get_nc():
    global _NC_CACHE
    if _NC_CACHE is None:
        _NC_CACHE = build_program()
    return _NC_CACHE


def kernel(input, weight):
    """input: [8,64,128,128] f32, weight: [8,8,9,16384] f32 ->
    [8,64,128,128] f32."""
    from concourse.bass_utils import run_bass_kernel_spmd

    x = np.ascontiguousarray(np.asarray(input, dtype=np.float32))
    w = np.ascontiguousarray(np.asarray(weight, dtype=np.float32))
    N = x.shape[0]
    nc = _get_nc()
    in_maps = [{"x": x[i].reshape(C, S), "w": w[i].reshape(CW, 9, S)}
               for i in range(N)]
    res = run_bass_kernel_spmd(nc, in_maps, core_ids=list(range(N)))
    out = np.stack([res.results[i]["out"].reshape(C, H, W) for i in range(N)])
    return out


# revision 20
# speedup vs baseline: 1.1592x; 1.1592x over previous
"""SAN aggregation kernel for Trainium2 (Bass/Tile), 8-core data-parallel.

Problem: out[n,c,h,w] = sum_k w[n, c//8, k, h*W+w] * xpad[n, c, h+dh(k), w+dw(k)]
  x: [8, 64, 128, 128] f32, w: [8, 8, 9, 16384] f32, 3x3 window, pad 1.

Sharding: batch dim N=8 across 8 NeuronCores (1 image per core, no
cross-core communication).

Per-core layout (everything resident in SBUF):
  partitions p = hb*8 + cw   (hb: 16 row-blocks of 8 rows, cw: 8 weight chans)
  x_sb  [128, 8*10*130]: per (gl): rows [hb*8-1, hb*8+9) of channel
        c=cw*8+gl, each row stored with 130-px pitch (1 left + 128 + 1 right
        zero pad).  Shift (dh,dw) == flat offset dh*130+dw.
  w_sb  [128, 9*1024]:  w[cw, k, hb-rows] per partition, k-major.

Compute: per gl, one (or nine) DVE tensor_mul producing all 9 k-product
planes into a q tile, then the 9-way k-reduction either on DVE
(tensor_reduce axis=X, k-innermost q layout) or on GPSIMD (pairwise
tensor_add tree, k-major q layout).  GPSIMD runs concurrently with DVE
(tensor ops use only DVE's dedicated SBUF port pair).
"""

import sys
import os

for _p in ("/opt/trn_rl_repo", "/root/.axon_site/_ro/trn_rl_repo"):
    if _p not in sys.path and os.path.isdir(_p):
        sys.path.append(_p)

import numpy as np

import concourse.bass as bass
import concourse.bacc as bacc
import concourse.mybir as mybir
import bass_rust
from concourse.tile import TileContext

F32 = mybir.dt.float32

C, H, W = 64, 128, 128
S = H * W          # 16384
CW, GL = 8, 8      # weight channels, share planes
HB = 16            # row blocks
RB = H // HB       # rows per block = 8
PITCH = W + 2      # 130
XROWS = RB + 2     # 10 rows incl halo
XGL = XROWS * PITCH  # 1300 elements per gl block in x_sb
SB = RB * W        # 1024 spatial elems per partition per gl

# gls whose k-reduction runs on DVE (tensor_reduce); the rest use a GPSIMD
# add tree.  DVE-reduce gls are placed last so GPSIMD ramps up first.
N_DVE_REDUCE = 2


def _ap(base, dims, extra_offset=0):
    """Copy AP `base`, replace its [step,count] dims, bump offset.

    dims[0] is the partition dim: step "P" substitutes the base AP's own
    partition stride (flat element space, = free width).
    """
    c = base.copy()
    pstep = base.ap[0][0]
    dims = [[pstep if s == "P" else s, n] for s, n in dims]
    c.ap = bass_rust.VecI64Pair(dims)
    if extra_offset:
        c.offset = c.offset + extra_offset
    return c


def build_program(n_dve_reduce=N_DVE_REDUCE):
    nc = bacc.Bacc("TRN2", target_bir_lowering=False, debug=False)
    x_d = nc.dram_tensor("x", [C, S], F32, kind="ExternalInput")
    w_d = nc.dram_tensor("w", [CW, 9, S], F32, kind="ExternalInput")
    o_d = nc.dram_tensor("out", [C, S], F32, kind="ExternalOutput")

    with TileContext(nc) as tc:
        with tc.tile_pool(name="main", bufs=1) as pool, \
             tc.tile_pool(name="qs", bufs=2) as qpool, \
             tc.tile_pool(name="stg", bufs=8) as spool, \
             tc.tile_pool(name="os", bufs=8) as opool:
            x_sb = pool.tile([128, GL * XGL], F32)
            w_sb = pool.tile([128, 9 * SB], F32)

            # ---- zero x_sb padding (left/right cols, halo rows).  Halo
            # rows zeroed on ALL partitions; r=0/r=9 DMAs overwrite the
            # valid parts (Tile orders the WAW deps).
            nc.vector.memset(
                _ap(x_sb[:], [["P", 128], [XGL, GL], [PITCH, RB],
                              [PITCH - 1, 2]], extra_offset=PITCH), 0.0)
            nc.vector.memset(
                _ap(x_sb[:], [["P", 128], [XGL, GL], [1, PITCH]]), 0.0)
            nc.vector.memset(
                _ap(x_sb[:], [["P", 128], [XGL, GL], [1, PITCH]],
                    extra_offset=(XROWS - 1) * PITCH), 0.0)

            # ---- x load: per gl, big contiguous HBM DMAs into a
            # 128-pitch staging tile (stage[p, r*128+j] = x row hb*8-1+r),
            # then one SBUF->SBUF DMA re-layout into the 130-pitch padded
            # x_sb block.  Big DMAs keep HBM near peak bandwidth.
            def load_w_k(k):
                nc.sync.dma_start(
                    out=_ap(w_sb[:], [["P", 128], [1, SB]],
                            extra_offset=k * SB),
                    in_=_ap(w_d.ap(), [[SB, HB], [9 * S, CW], [1, SB]],
                            extra_offset=k * S))

            def load_x_gl(gl, stage):
                # main: partitions 8..120 (hb 1..14), rows hb*8-1 .. hb*8+9
                # = 1280 contiguous elements of channel c starting at
                # (hb*8-1)*128.
                nc.sync.dma_start(
                    out=_ap(stage[8:120], [["P", 112], [1, XROWS * W]]),
                    in_=_ap(x_d.ap(), [[RB * W, HB - 2], [GL * S, CW],
                                       [1, XROWS * W]],
                            extra_offset=gl * S + (RB - 1) * W))
                # hb=0 (partitions 0..8): stage rows r=1..9 = x rows 0..8
                nc.scalar.dma_start(
                    out=_ap(stage[0:8], [["P", 8], [1, (XROWS - 1) * W]],
                            extra_offset=W),
                    in_=_ap(x_d.ap(), [[GL * S, CW], [1, (XROWS - 1) * W]],
                            extra_offset=gl * S))
                # hb=15 (partitions 120..128): stage rows r=0..8 = x rows
                # 119..127
                nc.scalar.dma_start(
                    out=_ap(stage[120:128], [["P", 8], [1, (XROWS - 1) * W]]),
                    in_=_ap(x_d.ap(), [[GL * S, CW], [1, (XROWS - 1) * W]],
                            extra_offset=gl * S + (H - XROWS + 1) * W))
                # re-layout 128-pitch -> 130-pitch (cols 1..129), skipping
                # the zeroed halo rows at the hb=0 / hb=15 edges.
                nc.sync.dma_start(
                    out=_ap(x_sb[8:120], [["P", 112], [PITCH, XROWS], [1, W]],
                            extra_offset=gl * XGL + 1),
                    in_=_ap(stage[8:120], [["P", 112], [W, XROWS], [1, W]]))
                nc.scalar.dma_start(
                    out=_ap(x_sb[0:8], [["P", 8], [PITCH, XROWS - 1], [1, W]],
                            extra_offset=gl * XGL + PITCH + 1),
                    in_=_ap(stage[0:8], [["P", 8], [W, XROWS - 1], [1, W]],
                            extra_offset=W))
                nc.scalar.dma_start(
                    out=_ap(x_sb[120:128], [["P", 8], [PITCH, XROWS - 1],
                                            [1, W]],
                            extra_offset=gl * XGL + 1),
                    in_=_ap(stage[120:128], [["P", 8], [W, XROWS - 1],
                                             [1, W]]))

            load_w_k(0)
            load_w_k(1)
            stages = []
            for gl in range(GL):
                stage = spool.tile([128, XROWS * W], F32, tag="stage")
                load_x_gl(gl, stage)
                if gl == 0:
                    for k in range(2, 9):
                        load_w_k(k)

            # ---- compute ----
            def xv(gl, dh, dw):
                return _ap(x_sb[:], [["P", 128], [PITCH, RB], [1, W]],
                           extra_offset=gl * XGL + dh * PITCH + dw)

            def mult_all_k(gl, q, kmajor):
                """Nine tensor_muls producing the 9 k-product planes.
                (DVE ISA APs are TENSOR3D: partition + 2 free dims max.)"""
                for k in range(9):
                    dh, dw = divmod(k, 3)
                    wv = _ap(w_sb[:], [["P", 128], [W, RB], [1, W]],
                             extra_offset=k * SB)
                    if kmajor:   # q[p, k*SB + s]
                        qv = _ap(q[:], [["P", 128], [W, RB], [1, W]],
                                 extra_offset=k * SB)
                    else:        # q[p, s*9 + k]
                        qv = _ap(q[:], [["P", 128], [9 * W, RB], [9, W]],
                                 extra_offset=k)
                    nc.vector.tensor_mul(out=qv, in0=xv(gl, dh, dw), in1=wv)

            def out_dma(gl, src):
                nc.scalar.dma_start(
                    out=_ap(o_d.ap(), [[RB * W, HB], [GL * S, CW], [1, SB]],
                            extra_offset=gl * S),
                    in_=src)

            # All compute on DVE: GPSIMD streaming concurrently with DVE
            # measurably slows DVE ~2.6x (shared SBUF port), so offloading
            # the reduction is a net loss.  Per gl: 9 muls + 8 adds, then
            # the output DMA (per-gl so stores overlap later compute).
            for gl in range(GL):
                acc_t = opool.tile([128, SB], F32, tag="o")
                acc = _ap(acc_t[:], [["P", 128], [W, RB], [1, W]])
                for k in range(9):
                    dh, dw = divmod(k, 3)
                    wv = _ap(w_sb[:], [["P", 128], [W, RB], [1, W]],
                             extra_offset=k * SB)
                    if k == 0:
                        nc.vector.tensor_mul(out=acc, in0=xv(gl, dh, dw),
                                             in1=wv)
                    else:
                        tmp = qpool.tile([128, SB], F32, tag="tmp")
                        t = _ap(tmp[:], [["P", 128], [W, RB], [1, W]])
                        nc.vector.tensor_mul(out=t, in0=xv(gl, dh, dw),
                                             in1=wv)
                        nc.vector.tensor_add(out=acc, in0=acc, in1=t)
                out_dma(gl, _ap(acc_t[:], [["P", 128], [1, SB]]))

    nc.compile()
    return nc


_NC_CACHE = None


def _get_nc():
    global _NC_CACHE
    if _NC_CACHE is None:
        _NC_CACHE = build_program()
    return _NC_CACHE


def kernel(input, weight):
    """input: [8,64,128,128] f32, weight: [8,8,9,16384] f32 ->
    [8,64,128,128] f32."""
    from concourse.bass_utils import run_bass_kernel_spmd

    x = np.ascontiguousarray(np.asarray(input, dtype=np.float32))
    w = np.ascontiguousarray(np.asarray(weight, dtype=np.float32))
    N = x.shape[0]
    nc = _get_nc()
    in_maps = [{"x": x[i].reshape(C, S), "w": w[i].reshape(CW, 9, S)}
               for i in range(N)]
    res = run_bass_kernel_spmd(nc, in_maps, core_ids=list(range(N)))
    out = np.stack([res.results[i]["out"].reshape(C, H, W) for i in range(N)])
    return out
